# revision 19
# baseline (speedup 1.0000x reference)
"""HaciCognitiveNet Trainium2 kernel, v3 (centered activations, packed stats).

Data-parallel over batch: B=8 -> one batch element per NeuronCore.
Activations live TRANSPOSED on-chip ([D, S], D on partitions).

Changes vs v2.1:
  - Rank-1 mean-correction matmuls (16/layer, 512 rows each) are GONE.
    The pre-LN mean is subtracted once: hc[c] = ht[c] + (-mu) broadcast,
    4 DVE ops/layer, and all four Q/K/V/G projections consume the
    centered hc tiles (V uses them as lhsT, so it centers for free).
  - LN stats (sums/ssq then sums2/ssq2) accumulate into ONE PSUM bank
    at partitions 0/32 (inner-LN rows reuse the pre-LN rows after their
    last read; M=1 matmuls with tile_position cols), freeing
    a bank: the main psum pool grows to 6 bufs, easing the Q->K->V->G
    drain backpressure that stalled the PE.
  - Stats matmuls for block l+1 issue during block l's residual drains
    (sums in set_ht); the ssq matmuls are deferred until after the next
    layer's Q mains so the PE never waits on the squares.
  - Squares for ssq run on the otherwise-idle Pool engine; rstd/gate/nm2
    planes broadcast there too (gpsimd.partition_broadcast) instead of
    PE matmul + Act drain. Only the latency-critical negmu plane keeps
    the K=1 PE broadcast (rpsum rotation: plane -> rskp -> rp x4).
  - bf16 residual shadows (ht_b) only exist where the FFN consumes them
    (after wm retentions); everywhere else DVE reads the f32r residual.
  - Drains split across engines: Q + G-premult + O + score diagonals on
    DVE, K/V/G-sigmoid/ret/score off-diagonals on Act.
"""

import numpy as np

B, S, DIN, D, H, FF = 8, 512, 384, 512, 8, 2048
DH = D // H
N_WM, N_CORE = 2, 4
NL = N_WM + N_CORE
DECAY = 0.99
EPS = 1e-5
PT = D // 128   # 4 partition tiles of the model dim
CT = S // 128   # 4 tiles of the sequence dim

_CACHE = {}


def _lhsT_layout(w):
    """[K, M] weight -> SBUF lhsT tile layout [128, (K//128)*M]."""
    k, m = w.shape
    c = k // 128
    return np.ascontiguousarray(
        w.reshape(c, 128, m).transpose(1, 0, 2).reshape(128, c * m)
    ).astype(np.float32)


def _build_program():
    import concourse.bass as bass
    import concourse.tile as tile
    from concourse import mybir, bacc
    from contextlib import ExitStack

    f32 = mybir.dt.float32
    f32r = mybir.dt.float32r
    bf16 = mybir.dt.bfloat16
    AF = mybir.ActivationFunctionType
    ALU = mybir.AluOpType

    nc = bacc.Bacc("TRN2", target_bir_lowering=False, debug=False)

    # Make Ln and Exp resolve to the single combined table set so each
    # LayerNorm chain pays one ACT_TABLE_LOAD instead of two.
    from concourse.hw_specs import get_activation_tables
    _tabs = get_activation_tables(nc.m.arch)
    for _name, _set in _tabs.items():
        if _name != "natural_log_exp_and_others":
            _set.discard(AF.Ln)
            _set.discard(AF.Exp)

    XT = nc.dram_tensor("xt", [128, 3, 512], bf16, kind="ExternalInput").ap()
    INW = nc.dram_tensor("inw", [128, 3 * 512], bf16, kind="ExternalInput").ap()
    INB = nc.dram_tensor("inb", [128, 4], f32, kind="ExternalInput").ap()
    WST = nc.dram_tensor("wst", [NL, 5, 128, 2048], bf16, kind="ExternalInput").ap()
    BG = nc.dram_tensor("bg", [NL, 128, 4], f32, kind="ExternalInput").ap()
    BO = nc.dram_tensor("bo", [NL, 128, 4], f32, kind="ExternalInput").ap()
    W1T = nc.dram_tensor("w1t", [128, 4, 2048], bf16, kind="ExternalInput").ap()
    W2T = nc.dram_tensor("w2t", [128, 16 * 512], bf16, kind="ExternalInput").ap()
    B1C = nc.dram_tensor("b1c", [128, 16], f32, kind="ExternalInput").ap()
    B2C = nc.dram_tensor("b2c", [128, 4], f32, kind="ExternalInput").ap()
    DK = nc.dram_tensor("dk", [1, 512], bf16, kind="ExternalInput").ap()
    DKC = nc.dram_tensor("dkc", [128, 4], f32, kind="ExternalInput").ap()
    MSKD = nc.dram_tensor("mskd", [128, 128], f32, kind="ExternalInput").ap()
    ONESC = nc.dram_tensor("onesc", [128, 1], bf16, kind="ExternalInput").ap()
    ONESF = nc.dram_tensor("onesf", [128, 1], f32r, kind="ExternalInput").ap()
    ONESR = nc.dram_tensor("onesr", [1, 128], bf16, kind="ExternalInput").ap()
    HOUT = nc.dram_tensor("hout", [4, 128, 512], f32r, kind="ExternalOutput").ap()

    with tile.TileContext(nc) as tc:
        with ExitStack() as ctx:
            consts = ctx.enter_context(tc.tile_pool(name="consts", bufs=1))
            wpool = ctx.enter_context(tc.tile_pool(name="wpool", bufs=10))
            w2pool = ctx.enter_context(tc.tile_pool(name="w2pool", bufs=4))
            wsmall = ctx.enter_context(tc.tile_pool(name="wsmall", bufs=2))
            hpool = ctx.enter_context(tc.tile_pool(name="hpool", bufs=2))
            hbpool = ctx.enter_context(tc.tile_pool(name="hbpool", bufs=2))
            hcpool = ctx.enter_context(tc.tile_pool(name="hcpool", bufs=2))
            apool = ctx.enter_context(tc.tile_pool(name="apool", bufs=1))
            atpool = ctx.enter_context(tc.tile_pool(name="atpool", bufs=8))
            spool = ctx.enter_context(tc.tile_pool(name="spool", bufs=8))
            sqpool = ctx.enter_context(tc.tile_pool(name="sqpool", bufs=2))
            grpool = ctx.enter_context(tc.tile_pool(name="grpool", bufs=5))
            retpool = ctx.enter_context(tc.tile_pool(name="retpool", bufs=1))
            plpool = ctx.enter_context(tc.tile_pool(name="plpool", bufs=3))
            odpool = ctx.enter_context(tc.tile_pool(name="odpool", bufs=2))
            f1pool = ctx.enter_context(tc.tile_pool(name="f1pool", bufs=3))
            psum = ctx.enter_context(tc.tile_pool(name="psum", bufs=6, space="PSUM"))
            stps = ctx.enter_context(tc.tile_pool(name="stps", bufs=1, space="PSUM"))
            rpsum = ctx.enter_context(tc.tile_pool(name="rpsum", bufs=1, space="PSUM"))

            ht = [None] * PT
            ht_b = [None] * PT
            stats = {"cur": None, "cur_sq": None, "nxt": None, "nxt_sq": None}

            def set_ht(j, hn, shadow=False, want_stats=True):
                """Residual tile update + next-block LN stats (sums on PE,
                squares on the Pool engine; ssq matmuls come later)."""
                ht[j] = hn
                if shadow or want_stats:
                    # bf16 shadow on the Act engine (slack at the O tail);
                    # feeds the FFN mains and/or the bf16 sums matmuls, so
                    # no f32r self-loading weight ever stalls the PE.
                    hb = hbpool.tile([128, 512], bf16, tag=f"htb{j}")
                    nc.scalar.copy(hb[:], hn[:])
                    ht_b[j] = hb
                if want_stats:
                    if j == 0:
                        stats["nxt"] = stps.tile([128, 512], f32, tag="st",
                                                 name="stats")
                        stats["nxt_sq"] = [None] * PT
                        stats["nxt_hn"] = [None] * PT
                    stats["nxt_hn"][j] = ht_b[j]
                    sq = sqpool.tile([128, 512], bf16, tag=f"sqs{j}")
                    nc.vector.tensor_mul(sq[:], hn[:], hn[:])
                    stats["nxt_sq"][j] = sq

            def issue_sums():
                """bf16 sums, batched consecutively after the residual loop."""
                for j in range(PT):
                    nc.tensor.matmul(stats["nxt"][0:1, :], onesc_sb[:],
                                     stats["nxt_hn"][j][:],
                                     start=(j == 0), stop=(j == PT - 1),
                                     skip_group_check=True)

            def roll_stats():
                stats["cur"] = stats["nxt"]
                stats["cur_sq"] = stats["nxt_sq"]
                stats["nxt"] = None
                stats["nxt_sq"] = None

            def issue_ssq():
                """ssq matmuls into the current stats tile, partition 32."""
                st = stats["cur"]
                for j in range(PT):
                    nc.tensor.matmul(st[32:33, :], onesc_sb[:],
                                     stats["cur_sq"][j][:],
                                     start=(j == 0), stop=(j == PT - 1),
                                     skip_group_check=True)

            def fetch_weights(lidx):
                wmat = []
                for i in range(5):
                    wt = wpool.tile([128, 2048], bf16, tag="wmat",
                                    name=f"wm{lidx}_{i}")
                    nc.sync.dma_start(out=wt[:], in_=WST[lidx, i])
                    wmat.append(wt)
                return wmat

            # ---- input projection: ht = (x @ in_w + in_b)^T ----
            inctx = ExitStack()
            inpool = inctx.enter_context(tc.tile_pool(name="inpool", bufs=1))
            xt_sb = inpool.tile([128, 3, 512], bf16)
            nc.sync.dma_start(out=xt_sb[:], in_=XT[:])
            inw_sb = inpool.tile([128, 3 * 512], bf16)
            nc.sync.dma_start(out=inw_sb[:], in_=INW[:])
            inb_sb = inpool.tile([128, 4], f32)
            nc.sync.dma_start(out=inb_sb[:], in_=INB[:])
            # ---- consts ----
            dk_sb = consts.tile([1, 512], bf16)
            nc.sync.dma_start(out=dk_sb[:], in_=DK[:])
            mskd_sb = consts.tile([128, 128], f32)
            nc.sync.dma_start(out=mskd_sb[:], in_=MSKD[:])
            onesc_sb = consts.tile([128, 1], bf16)
            nc.sync.dma_start(out=onesc_sb[:], in_=ONESC[:])
            onesf_sb = consts.tile([128, 1], f32r)
            nc.sync.dma_start(out=onesf_sb[:], in_=ONESF[:])
            onesr_sb = consts.tile([1, 128], bf16)
            nc.sync.dma_start(out=onesr_sb[:], in_=ONESR[:])
            dkc_sb = consts.tile([128, 4], f32)
            nc.sync.dma_start(out=dkc_sb[:], in_=DKC[:])
            b1_sb = consts.tile([128, 16], f32)
            nc.sync.dma_start(out=b1_sb[:], in_=B1C[:])
            b2_sb = consts.tile([128, 4], f32)
            nc.sync.dma_start(out=b2_sb[:], in_=B2C[:])

            for j in range(PT):
                p = psum.tile([128, 512], f32, tag="big")
                for c in range(3):
                    nc.tensor.matmul(
                        p[:], inw_sb[:, c * 512 + 128 * j : c * 512 + 128 * (j + 1)],
                        xt_sb[:, c, :], start=(c == 0), stop=(c == 2))
                hj = hpool.tile([128, 512], f32r, tag=f"ht{j}")
                nc.scalar.activation(hj[:], p[:], AF.Identity, bias=inb_sb[:, j : j + 1])
                set_ht(j, hj)
            issue_sums()
            inctx.close()

            def rstd_row(ssq_ps, negmu_b, name):
                """rstd = Exp(-0.5*Ln(ssq/D - mu^2 + eps)) -> bf16 [1,512]."""
                m2 = spool.tile([1, 512], f32, tag="tiny", name=f"m2{name}")
                nc.vector.tensor_mul(m2[:], negmu_b[:], negmu_b[:])
                m2e = spool.tile([1, 512], f32, tag="tiny", name=f"m2e{name}")
                nc.vector.tensor_scalar(m2e[:], m2[:], 1.0, -EPS, ALU.mult, ALU.add)
                w32 = spool.tile([1, 512], f32, tag="tiny", name=f"w32{name}")
                nc.vector.scalar_tensor_tensor(w32[:], ssq_ps[:], 1.0 / D, m2e[:],
                                               ALU.mult, ALU.subtract)
                lnw = spool.tile([1, 512], f32, tag="tiny", name=f"lnw{name}")
                nc.scalar.activation(lnw[:], w32[:], AF.Ln)
                r = spool.tile([1, 512], bf16, tag="tiny", name=f"r{name}")
                nc.scalar.activation(r[:], lnw[:], AF.Exp, scale=-0.5)
                return r

            def pbcast(row, name):
                """[1,512] bf16 row -> [128,512] bf16 plane on the Pool engine."""
                pl = plpool.tile([128, 512], bf16, tag="plane", name=f"pl{name}")
                nc.gpsimd.partition_broadcast(pl[:], row[:])
                return pl

            def retention(lidx, last=False, pre_hc=None):
                if pre_hc is None:
                    roll_stats()
                    st = stats["cur"]
                else:
                    # input is already mean-0/var-1 (post final LN): the
                    # pre-LN is the identity; only inner-LN stats live here.
                    st = stps.tile([128, 512], f32, tag="st", name="stats")
                wmat = fetch_weights(lidx)
                w1_pre = None
                if lidx < N_WM:
                    # prefetch the FFN up-projection during the retention so
                    # its first LDWEIGHTS never waits on HBM
                    w1_pre = []
                    for c in range(PT):
                        wt = wpool.tile([128, 2048], bf16, tag="wmat",
                                        name=f"w1_{c}")
                        nc.sync.dma_start(out=wt[:], in_=W1T[:, c, :])
                        w1_pre.append(wt)
                bg_sb = wsmall.tile([128, 4], f32, tag="bgc")
                nc.sync.dma_start(out=bg_sb[:], in_=BG[lidx])
                bo_sb = wsmall.tile([128, 4], f32, tag="boc")
                nc.sync.dma_start(out=bo_sb[:], in_=BO[lidx])

                g_b = None
                if pre_hc is None:
                    # pre-LN: negmu from packed sums, center residual into hc
                    negmu = spool.tile([1, 512], bf16, tag="tiny", name="negmu")
                    nc.vector.tensor_scalar_mul(negmu[:], st[0:1, :], -1.0 / D)
                    plane = rpsum.tile([128, 512], f32, tag="rp", name="nmplane")
                    nc.tensor.matmul(plane[:], onesr_sb[:], negmu[:],
                                     start=True, stop=True)
                    hc = []
                    for c in range(PT):
                        t = hcpool.tile([128, 512], bf16, tag=f"hc{c}")
                        nc.vector.tensor_add(t[:], ht[c][:], plane[:])
                        hc.append(t)
                    rsk_sb = spool.tile([128, 4], f32, tag="rsk")
                else:
                    hc = pre_hc
                    rsk_sb = dkc_sb

                qt, kt, vn, gt = [], [], [], []

                def proj_block(ip, dest):
                    pss = [psum.tile([128, 512], f32, tag="big", name=f"pss{ip}_{i}")
                           for i in range(PT)]
                    for c in range(PT):
                        for j in range(PT):
                            if ip == 2:
                                nc.tensor.matmul(
                                    pss[j][:], hc[c][:, 128 * j : 128 * (j + 1)],
                                    wmat[2][:, c * 512 : (c + 1) * 512],
                                    start=(c == 0), stop=(c == PT - 1))
                            else:
                                nc.tensor.matmul(
                                    pss[j][:],
                                    wmat[ip][:, c * 512 + 128 * j : c * 512 + 128 * (j + 1)],
                                    hc[c][:], start=(c == 0), stop=(c == PT - 1))
                    for j in range(PT):
                        t = apool.tile([128, 512], bf16, tag=f"proj{ip}_{j}")
                        if ip == 0:
                            nc.vector.tensor_copy(t[:], pss[j][:])
                        elif ip == 1:
                            nc.scalar.copy(t[:], pss[j][:])
                        elif ip == 2:
                            nc.scalar.activation(t[:], pss[j][:], AF.Identity,
                                                 scale=rsk_sb[:, j : j + 1])
                        elif g_b is None:
                            nc.scalar.activation(t[:], pss[j][:], AF.Sigmoid,
                                                 bias=bg_sb[:, j : j + 1])
                        else:
                            tg = grpool.tile([128, 512], bf16, tag="gtmp")
                            nc.vector.tensor_mul(tg[:], pss[j][:], g_b[:])
                            nc.scalar.activation(t[:], tg[:], AF.Sigmoid,
                                                 bias=bg_sb[:, j : j + 1])
                        dest.append(t)

                proj_block(0, qt)
                if pre_hc is None:
                    # defer the ssq matmuls + rstd chain until the PE has the
                    # Q mains in flight; rsk transpose waits past K mains.
                    issue_ssq()
                    r = rstd_row(st[32:33, :], negmu, "pre")
                    r2 = spool.tile([1, 512], bf16, tag="tiny", name="r2")
                    nc.vector.tensor_mul(r2[:], r[:], r[:])
                    ks = spool.tile([1, 512], bf16, tag="tiny", name="ks")
                    nc.vector.tensor_mul(ks[:], r2[:], dk_sb[:])
                    g_b = pbcast(r, "gb")

                proj_block(1, kt)

                if pre_hc is None:
                    # ks row -> per-partition columns via K=1 transposes
                    rskp = rpsum.tile([128, 4], f32, tag="rp", name="rskp")
                    for j in range(PT):
                        nc.tensor.matmul(rskp[:, j : j + 1],
                                         ks[:, 128 * j : 128 * (j + 1)],
                                         onesr_sb[0:1, 0:1], start=True, stop=True)
                    nc.vector.tensor_copy(rsk_sb[:], rskp[:])

                proj_block(2, vn)
                proj_block(3, gt)

                # scores + AV, head pairs on row/col groups
                ret_sb = []
                for jt in range(PT):
                    rp = rpsum.tile([128, 512], f32, tag="rp", name=f"rp{jt}")
                    at_tiles = {}

                    def issue_sc(k_t):
                        cs = 128 * k_t
                        npr = 512 - cs
                        for hh in range(2):
                            r0 = 64 * hh
                            sc = psum.tile([128, 512], f32, tag="big",
                                           name=f"sc{k_t}_{hh}")
                            nc.tensor.matmul(
                                sc[:, 0:npr],
                                kt[jt][r0 : r0 + 64, cs : cs + 128],
                                qt[jt][r0 : r0 + 64, cs : 512],
                                start=True, stop=True)
                            at = atpool.tile([128, 512], bf16, tag="at")
                            nc.vector.tensor_mul(
                                at[:, 0:128], sc[:, 0:128], mskd_sb[:])
                            if npr > 128:
                                if k_t == 0:
                                    nc.vector.tensor_copy(at[:, 128:npr],
                                                          sc[:, 128:npr])
                                else:
                                    nc.scalar.copy(at[:, 128:npr], sc[:, 128:npr])
                            at_tiles[(hh, k_t)] = at

                    def issue_av(k_t):
                        cs = 128 * k_t
                        npr = 512 - cs
                        for hh in range(2):
                            h = 2 * jt + hh
                            nc.tensor.matmul(
                                rp[64 * hh : 64 * hh + 64, cs : 512],
                                vn[k_t][:, 64 * h : 64 * (h + 1)],
                                at_tiles[(hh, k_t)][:, 0:npr],
                                start=(k_t == 0), stop=(k_t == CT - 1),
                                skip_group_check=True)

                    # software pipeline: scores run two k-blocks ahead of the
                    # AV accumulation, giving the at drains slack and freeing
                    # score psum banks early.
                    issue_sc(0)
                    issue_sc(1)
                    issue_av(0)
                    issue_sc(2)
                    issue_av(1)
                    issue_sc(3)
                    issue_av(2)
                    issue_av(3)
                    rs = retpool.tile([128, 512], bf16, tag=f"ret{jt}")
                    nc.scalar.copy(rs[:], rp[:])
                    ret_sb.append(rs)
                    s2 = sqpool.tile([128, 512], bf16, tag=f"sq2{jt}")
                    nc.vector.tensor_mul(s2[:], rs[:], rs[:])
                    nc.tensor.matmul(st[0:1, :], onesc_sb[:], rs[:],
                                     start=(jt == 0), stop=(jt == PT - 1),
                                     skip_group_check=True)
                    nc.tensor.matmul(st[32:33, :], onesc_sb[:], s2[:],
                                     start=(jt == 0), stop=(jt == PT - 1),
                                     skip_group_check=True)

                # inner LN
                negmu2 = spool.tile([1, 512], bf16, tag="tiny", name="negmu2")
                nc.vector.tensor_scalar_mul(negmu2[:], st[0:1, :], -1.0 / D)
                nm2p = rpsum.tile([128, 512], f32, tag="rp", name="nm2p")
                nc.tensor.matmul(nm2p[:], onesr_sb[:], negmu2[:],
                                 start=True, stop=True)
                rB = rstd_row(st[32:33, :], negmu2, "inn")
                rstd2_b = pbcast(rB, "rstd2")

                gret = []
                for j in range(PT):
                    tmpc = odpool.tile([128, 512], bf16, tag="odb")
                    nc.vector.tensor_add(tmpc[:], ret_sb[j][:], nm2p[:])
                    gr = grpool.tile([128, 512], bf16, tag="gret")
                    nc.vector.tensor_mul(gr[:], tmpc[:], gt[j][:])
                    gret.append(gr)

                # O mains + rstd2/bias/residual, j-outer; next-block stats
                # (sums + Pool squares) interleave via set_ht.
                want_stats = (not last) and (lidx >= N_WM)
                shadow = lidx < N_WM
                for j in range(PT):
                    p1 = psum.tile([128, 512], f32, tag="big", name=f"p1s{j}")
                    for c in range(PT):
                        nc.tensor.matmul(
                            p1[:],
                            wmat[4][:, c * 512 + 128 * j : c * 512 + 128 * (j + 1)],
                            gret[c][:], start=(c == 0), stop=(c == PT - 1))
                    a = odpool.tile([128, 512], f32, tag="oda")
                    nc.vector.tensor_mul(a[:], p1[:], rstd2_b[:])
                    hn = hpool.tile([128, 512], f32r, tag=f"ht{j}")
                    nc.vector.scalar_tensor_tensor(hn[:], a[:], bo_sb[:, j : j + 1],
                                                   ht[j][:], ALU.add, ALU.add)
                    set_ht(j, hn, shadow=shadow, want_stats=want_stats)
                if want_stats:
                    issue_sums()
                return w1_pre

            def ffn(w1_sb):
                # stats for the successor block were accumulated by the
                # preceding retention's set_ht; this FFN does not read them.
                f2ps = [psum.tile([128, 512], f32, tag="big", name=f"f2ps{i}")
                        for i in range(PT)]
                for t in range(16):
                    p = psum.tile([128, 512], f32, tag="big")
                    for c in range(PT):
                        nc.tensor.matmul(
                            p[:], w1_sb[c][:, 128 * t : 128 * (t + 1)], ht_b[c][:],
                            start=(c == 0), stop=(c == PT - 1))
                    f1 = f1pool.tile([128, 512], bf16, tag="f1")
                    nc.scalar.activation(f1[:], p[:], AF.Gelu, bias=b1_sb[:, t : t + 1])
                    w2s = w2pool.tile([128, 512], bf16, tag="w2s")
                    nc.sync.dma_start(out=w2s[:], in_=W2T[:, t * 512 : (t + 1) * 512])
                    for j in range(PT):
                        nc.tensor.matmul(
                            f2ps[j][:], w2s[:, 128 * j : 128 * (j + 1)],
                            f1[:], start=(t == 0), stop=(t == 15))
                for j in range(PT):
                    hn = hpool.tile([128, 512], f32r, tag=f"ht{j}")
                    nc.vector.scalar_tensor_tensor(hn[:], f2ps[j][:], b2_sb[:, j : j + 1],
                                                   ht[j][:], ALU.add, ALU.add)
                    set_ht(j, hn)
                issue_sums()

            # world model layers. A wm retention's own set_ht feeds the
            # stats of the block after the FFN, and the FFN's set_ht feeds
            # the next retention / final LN: stats flow block-to-block.
            for l in range(N_WM):
                w1p = retention(l)
                ffn(w1p)

            # final LN of world model (stats packed by the last ffn).
            # wm_onw == 1 and wm_onb == 0 (asserted host-side), so this is a
            # plain LayerNorm: its output is mean-0/var-1 and the first core
            # retention's pre-LN becomes the identity -> feed hc directly.
            roll_stats()
            st = stats["cur"]
            issue_ssq()
            negmuf = spool.tile([1, 512], bf16, tag="tiny", name="negmuf")
            nc.vector.tensor_scalar_mul(negmuf[:], st[0:1, :], -1.0 / D)
            rf = rstd_row(st[32:33, :], negmuf, "fin")
            nmr = spool.tile([1, 512], bf16, tag="tiny", name="nmr")
            nc.vector.tensor_mul(nmr[:], negmuf[:], rf[:])
            rfp = psum.tile([128, 512], f32, tag="big", name="rfplane")
            nc.tensor.matmul(rfp[:], onesr_sb[:], rf[:], start=True, stop=True)
            nmrp = psum.tile([128, 512], f32, tag="big", name="nmrplane")
            nc.tensor.matmul(nmrp[:], onesr_sb[:], nmr[:], start=True, stop=True)
            pre_hc = []
            for j in range(PT):
                t1 = odpool.tile([128, 512], f32, tag="oda")
                nc.vector.tensor_mul(t1[:], ht[j][:], rfp[:])
                hn = hpool.tile([128, 512], f32r, tag=f"ht{j}")
                nc.vector.tensor_add(hn[:], t1[:], nmrp[:])
                set_ht(j, hn, want_stats=False)
                hcb = hcpool.tile([128, 512], bf16, tag=f"hc{j}")
                nc.scalar.copy(hcb[:], hn[:])
                pre_hc.append(hcb)

            # retention core layers
            retention(N_WM, pre_hc=pre_hc)
            for l in range(N_WM + 1, NL):
                retention(l, last=(l == NL - 1))

            for j in range(PT):
                nc.sync.dma_start(out=HOUT[j], in_=ht[j][:])

    nc.compile()
    return nc


def _host_prep(inputs):
    """Fold weights host-side; returns the shared in_map dict (no xt)."""
    import ml_dtypes
    bf = ml_dtypes.bfloat16
    g = {k: np.asarray(v, dtype=np.float32) for k, v in inputs.items()}

    def layer_params(l):
        if l < N_WM:
            pre = "wm_"
            i = l
        else:
            pre = "co_"
            i = l - N_WM
        return {n: g[pre + n][i] for n in
                ("wq", "bq", "wk", "bk", "wv", "bv", "wg", "bg", "wo", "bo",
                 "lnw", "lnb", "prew", "preb")}

    wst = np.zeros((NL, 5, 128, 2048), np.float32)
    bgc = np.zeros((NL, 128, 4), np.float32)
    boc = np.zeros((NL, 128, 4), np.float32)
    for l in range(NL):
        p = layer_params(l)
        wq = p["prew"][:, None] * p["wq"]
        wk = p["prew"][:, None] * p["wk"]
        wv = p["prew"][:, None] * p["wv"]
        wg = p["prew"][:, None] * p["wg"]
        wo = p["lnw"][:, None] * p["wo"]
        # biases bq~ = bq + preb @ wq must be zero for this folded fast path
        for nm, w in (("bq", p["wq"]), ("bk", p["wk"]), ("bv", p["wv"])):
            bb = p[nm] + p["preb"] @ w
            assert np.abs(bb).max() == 0.0, f"nonzero {nm} not supported"
        assert np.abs(p["lnb"]).max() == 0.0, "nonzero lnb not supported"
        bgf = p["bg"] + p["preb"] @ p["wg"]
        wst[l, 0] = _lhsT_layout(wq)
        wst[l, 1] = _lhsT_layout(wk)
        wst[l, 2] = _lhsT_layout(wv)
        wst[l, 3] = _lhsT_layout(wg)
        wst[l, 4] = _lhsT_layout(wo)
        bgc[l] = bgf.reshape(4, 128).T
        boc[l] = p["bo"].reshape(4, 128).T

    inw = _lhsT_layout(g["in_w"])
    inb = g["in_b"].reshape(4, 128).T.copy()
    w1t = _lhsT_layout(g["ffn_w1"]).reshape(128, 4, 2048)
    w2t = _lhsT_layout(g["ffn_w2"])  # [128, 16*512]
    b1c = g["ffn_b1"].reshape(16, 128).T.copy()
    b2c = g["ffn_b2"].reshape(4, 128).T.copy()
    assert np.all(g["wm_onw"] == 1.0), "non-unit wm_onw not supported"
    assert np.all(g["wm_onb"] == 0.0), "nonzero wm_onb not supported"

    q = np.arange(S, dtype=np.float64)
    dk = (DECAY ** (-q)).astype(np.float32).reshape(1, 512)
    mskd = np.triu(np.ones((128, 128), np.float32))

    return {
        "inw": inw.astype(bf), "inb": inb, "wst": wst.astype(bf),
        "bg": bgc, "bo": boc,
        "w1t": np.ascontiguousarray(w1t).astype(bf), "w2t": w2t.astype(bf),
        "b1c": b1c, "b2c": b2c,
        "dk": dk.astype(bf), "dkc": dk.reshape(4, 128).T.copy(), "mskd": mskd,
        "onesc": np.ones((128, 1), np.float32).astype(bf),
        "onesf": np.ones((128, 1), np.float32),
        "onesr": np.ones((1, 128), np.float32).astype(bf),
    }


def kernel(**inputs):
    from concourse.bass_utils import run_bass_kernel_spmd
    import ml_dtypes

    if "nc" not in _CACHE:
        _CACHE["nc"] = _build_program()
    nc = _CACHE["nc"]

    shared = _host_prep(inputs)
    x = np.asarray(inputs["x"], dtype=np.float32)
    in_maps = []
    for b in range(B):
        xt = np.ascontiguousarray(
            x[b].T.reshape(3, 128, 512).transpose(1, 0, 2)).astype(ml_dtypes.bfloat16)
        m = dict(shared)
        m["xt"] = xt
        in_maps.append(m)

    res = run_bass_kernel_spmd(nc, in_maps, list(range(B)))
    out = np.empty((B, S, D), np.float32)
    for b in range(B):
        hout = res.results[b]["hout"]  # [4,128,512] = ht tiles (transposed h)
        out[b] = hout.reshape(512, 512).T
    return out


# revision 20
# speedup vs baseline: 1.0589x; 1.0589x over previous
"""HaciCognitiveNet Trainium2 kernel, v3 (centered activations, packed stats).

Data-parallel over batch: B=8 -> one batch element per NeuronCore.
Activations live TRANSPOSED on-chip ([D, S], D on partitions).

Changes vs v2.1:
  - Rank-1 mean-correction matmuls (16/layer, 512 rows each) are GONE.
    The pre-LN mean is subtracted once: hc[c] = ht[c] + (-mu) broadcast,
    4 DVE ops/layer, and all four Q/K/V/G projections consume the
    centered hc tiles (V uses them as lhsT, so it centers for free).
  - LN stats (sums/ssq then sums2/ssq2) accumulate into ONE PSUM bank
    at partitions 0/32 (inner-LN rows reuse the pre-LN rows after their
    last read; M=1 matmuls with tile_position cols), freeing
    a bank: the main psum pool grows to 6 bufs, easing the Q->K->V->G
    drain backpressure that stalled the PE.
  - Stats matmuls for block l+1 issue during block l's residual drains
    (sums in set_ht); the ssq matmuls are deferred until after the next
    layer's Q mains so the PE never waits on the squares.
  - Squares for ssq run on the otherwise-idle Pool engine; rstd/gate/nm2
    planes broadcast there too (gpsimd.partition_broadcast) instead of
    PE matmul + Act drain. Only the latency-critical negmu plane keeps
    the K=1 PE broadcast (rpsum rotation: plane -> rskp -> rp x4).
  - bf16 residual shadows (ht_b) only exist where the FFN consumes them
    (after wm retentions); everywhere else DVE reads the f32r residual.
  - Drains split across engines: Q + G-premult + O + score diagonals on
    DVE, K/V/G-sigmoid/ret/score off-diagonals on Act.
"""

import numpy as np

B, S, DIN, D, H, FF = 8, 512, 384, 512, 8, 2048
DH = D // H
N_WM, N_CORE = 2, 4
NL = N_WM + N_CORE
DECAY = 0.99
EPS = 1e-5
PT = D // 128   # 4 partition tiles of the model dim
CT = S // 128   # 4 tiles of the sequence dim

_CACHE = {}


def _lhsT_layout(w):
    """[K, M] weight -> SBUF lhsT tile layout [128, (K//128)*M]."""
    k, m = w.shape
    c = k // 128
    return np.ascontiguousarray(
        w.reshape(c, 128, m).transpose(1, 0, 2).reshape(128, c * m)
    ).astype(np.float32)


def _build_program():
    import concourse.bass as bass
    import concourse.tile as tile
    from concourse import mybir, bacc
    from contextlib import ExitStack

    f32 = mybir.dt.float32
    f32r = mybir.dt.float32r
    bf16 = mybir.dt.bfloat16
    AF = mybir.ActivationFunctionType
    ALU = mybir.AluOpType

    nc = bacc.Bacc("TRN2", target_bir_lowering=False, debug=False)

    # Make Ln and Exp resolve to the single combined table set so each
    # LayerNorm chain pays one ACT_TABLE_LOAD instead of two.
    from concourse.hw_specs import get_activation_tables
    _tabs = get_activation_tables(nc.m.arch)
    for _name, _set in _tabs.items():
        if _name != "natural_log_exp_and_others":
            _set.discard(AF.Ln)
            _set.discard(AF.Exp)

    XT = nc.dram_tensor("xt", [128, 3, 512], bf16, kind="ExternalInput").ap()
    INW = nc.dram_tensor("inw", [128, 3 * 512], bf16, kind="ExternalInput").ap()
    INB = nc.dram_tensor("inb", [128, 4], f32, kind="ExternalInput").ap()
    WST = nc.dram_tensor("wst", [NL, 5, 128, 2048], bf16, kind="ExternalInput").ap()
    BG = nc.dram_tensor("bg", [NL, 128, 4], f32, kind="ExternalInput").ap()
    BO = nc.dram_tensor("bo", [NL, 128, 4], f32, kind="ExternalInput").ap()
    W1T = nc.dram_tensor("w1t", [128, 4, 2048], bf16, kind="ExternalInput").ap()
    W2T = nc.dram_tensor("w2t", [128, 16 * 512], bf16, kind="ExternalInput").ap()
    B1C = nc.dram_tensor("b1c", [128, 16], f32, kind="ExternalInput").ap()
    B2C = nc.dram_tensor("b2c", [128, 4], f32, kind="ExternalInput").ap()
    DK = nc.dram_tensor("dk", [1, 512], bf16, kind="ExternalInput").ap()
    DKC = nc.dram_tensor("dkc", [128, 4], f32, kind="ExternalInput").ap()
    MSKD = nc.dram_tensor("mskd", [128, 128], f32, kind="ExternalInput").ap()
    ONESC = nc.dram_tensor("onesc", [128, 1], bf16, kind="ExternalInput").ap()
    ONESF = nc.dram_tensor("onesf", [128, 1], f32r, kind="ExternalInput").ap()
    ONESR = nc.dram_tensor("onesr", [1, 128], bf16, kind="ExternalInput").ap()
    HOUT = nc.dram_tensor("hout", [4, 128, 512], f32r, kind="ExternalOutput").ap()

    with tile.TileContext(nc) as tc:
        with ExitStack() as ctx:
            consts = ctx.enter_context(tc.tile_pool(name="consts", bufs=1))
            wpool = ctx.enter_context(tc.tile_pool(name="wpool", bufs=10))
            w2pool = ctx.enter_context(tc.tile_pool(name="w2pool", bufs=4))
            wsmall = ctx.enter_context(tc.tile_pool(name="wsmall", bufs=2))
            hpool = ctx.enter_context(tc.tile_pool(name="hpool", bufs=2))
            hbpool = ctx.enter_context(tc.tile_pool(name="hbpool", bufs=2))
            hcpool = ctx.enter_context(tc.tile_pool(name="hcpool", bufs=2))
            apool = ctx.enter_context(tc.tile_pool(name="apool", bufs=1))
            atpool = ctx.enter_context(tc.tile_pool(name="atpool", bufs=8))
            spool = ctx.enter_context(tc.tile_pool(name="spool", bufs=8))
            sqpool = ctx.enter_context(tc.tile_pool(name="sqpool", bufs=2))
            grpool = ctx.enter_context(tc.tile_pool(name="grpool", bufs=5))
            retpool = ctx.enter_context(tc.tile_pool(name="retpool", bufs=1))
            plpool = ctx.enter_context(tc.tile_pool(name="plpool", bufs=3))
            odpool = ctx.enter_context(tc.tile_pool(name="odpool", bufs=2))
            f1pool = ctx.enter_context(tc.tile_pool(name="f1pool", bufs=3))
            psum = ctx.enter_context(tc.tile_pool(name="psum", bufs=6, space="PSUM"))
            stps = ctx.enter_context(tc.tile_pool(name="stps", bufs=1, space="PSUM"))
            rpsum = ctx.enter_context(tc.tile_pool(name="rpsum", bufs=1, space="PSUM"))

            ht = [None] * PT
            ht_b = [None] * PT
            stats = {"cur": None, "cur_sq": None, "nxt": None, "nxt_sq": None}

            def set_ht(j, hn, shadow=False, want_stats=True):
                """Residual tile update + next-block LN stats (sums on PE,
                squares on the Pool engine; ssq matmuls come later)."""
                ht[j] = hn
                if shadow:
                    hb = hbpool.tile([128, 512], bf16, tag=f"htb{j}")
                    nc.scalar.copy(hb[:], hn[:])
                    ht_b[j] = hb
                if want_stats:
                    if j == 0:
                        stats["nxt"] = stps.tile([128, 512], f32, tag="st",
                                                 name="stats")
                        stats["nxt_sq"] = [None] * PT
                        stats["nxt_hn"] = [None] * PT
                    stats["nxt_hn"][j] = hn
                    sq = sqpool.tile([128, 512], bf16, tag=f"sqs{j}")
                    nc.vector.tensor_mul(sq[:], hn[:], hn[:])
                    stats["nxt_sq"][j] = sq

            def issue_sums():
                """f32r sums, batched consecutively after the residual loop:
                the slow self-loading f32r weight stalls once at the tail
                instead of 3-4 times inside the O-mains pipeline."""
                for j in range(PT):
                    nc.tensor.matmul(stats["nxt"][0:1, :], onesf_sb[:],
                                     stats["nxt_hn"][j][:],
                                     start=(j == 0), stop=(j == PT - 1),
                                     skip_group_check=True)

            def roll_stats():
                stats["cur"] = stats["nxt"]
                stats["cur_sq"] = stats["nxt_sq"]
                stats["nxt"] = None
                stats["nxt_sq"] = None

            def issue_ssq():
                """ssq matmuls into the current stats tile, partition 32."""
                st = stats["cur"]
                for j in range(PT):
                    nc.tensor.matmul(st[32:33, :], onesc_sb[:],
                                     stats["cur_sq"][j][:],
                                     start=(j == 0), stop=(j == PT - 1),
                                     skip_group_check=True)

            def fetch_weights(lidx):
                wmat = []
                for i in range(5):
                    wt = wpool.tile([128, 2048], bf16, tag="wmat",
                                    name=f"wm{lidx}_{i}")
                    nc.sync.dma_start(out=wt[:], in_=WST[lidx, i])
                    wmat.append(wt)
                return wmat

            # ---- input projection: ht = (x @ in_w + in_b)^T ----
            inctx = ExitStack()
            inpool = inctx.enter_context(tc.tile_pool(name="inpool", bufs=1))
            xt_sb = inpool.tile([128, 3, 512], bf16)
            nc.sync.dma_start(out=xt_sb[:], in_=XT[:])
            inw_sb = inpool.tile([128, 3 * 512], bf16)
            nc.sync.dma_start(out=inw_sb[:], in_=INW[:])
            inb_sb = inpool.tile([128, 4], f32)
            nc.sync.dma_start(out=inb_sb[:], in_=INB[:])
            # ---- consts ----
            dk_sb = consts.tile([1, 512], bf16)
            nc.sync.dma_start(out=dk_sb[:], in_=DK[:])
            mskd_sb = consts.tile([128, 128], f32)
            nc.sync.dma_start(out=mskd_sb[:], in_=MSKD[:])
            onesc_sb = consts.tile([128, 1], bf16)
            nc.sync.dma_start(out=onesc_sb[:], in_=ONESC[:])
            onesf_sb = consts.tile([128, 1], f32r)
            nc.sync.dma_start(out=onesf_sb[:], in_=ONESF[:])
            onesr_sb = consts.tile([1, 128], bf16)
            nc.sync.dma_start(out=onesr_sb[:], in_=ONESR[:])
            dkc_sb = consts.tile([128, 4], f32)
            nc.sync.dma_start(out=dkc_sb[:], in_=DKC[:])
            b1_sb = consts.tile([128, 16], f32)
            nc.sync.dma_start(out=b1_sb[:], in_=B1C[:])
            b2_sb = consts.tile([128, 4], f32)
            nc.sync.dma_start(out=b2_sb[:], in_=B2C[:])

            for j in range(PT):
                p = psum.tile([128, 512], f32, tag="big")
                for c in range(3):
                    nc.tensor.matmul(
                        p[:], inw_sb[:, c * 512 + 128 * j : c * 512 + 128 * (j + 1)],
                        xt_sb[:, c, :], start=(c == 0), stop=(c == 2))
                hj = hpool.tile([128, 512], f32r, tag=f"ht{j}")
                nc.scalar.activation(hj[:], p[:], AF.Identity, bias=inb_sb[:, j : j + 1])
                set_ht(j, hj)
            issue_sums()
            inctx.close()

            def rstd_row(ssq_ps, negmu_b, name):
                """rstd = Exp(-0.5*Ln(ssq/D - mu^2 + eps)) -> bf16 [1,512]."""
                m2 = spool.tile([1, 512], f32, tag="tiny", name=f"m2{name}")
                nc.vector.tensor_mul(m2[:], negmu_b[:], negmu_b[:])
                m2e = spool.tile([1, 512], f32, tag="tiny", name=f"m2e{name}")
                nc.vector.tensor_scalar(m2e[:], m2[:], 1.0, -EPS, ALU.mult, ALU.add)
                w32 = spool.tile([1, 512], f32, tag="tiny", name=f"w32{name}")
                nc.vector.scalar_tensor_tensor(w32[:], ssq_ps[:], 1.0 / D, m2e[:],
                                               ALU.mult, ALU.subtract)
                lnw = spool.tile([1, 512], f32, tag="tiny", name=f"lnw{name}")
                nc.scalar.activation(lnw[:], w32[:], AF.Ln)
                r = spool.tile([1, 512], bf16, tag="tiny", name=f"r{name}")
                nc.scalar.activation(r[:], lnw[:], AF.Exp, scale=-0.5)
                return r

            def pbcast(row, name):
                """[1,512] bf16 row -> [128,512] bf16 plane on the Pool engine."""
                pl = plpool.tile([128, 512], bf16, tag="plane", name=f"pl{name}")
                nc.gpsimd.partition_broadcast(pl[:], row[:])
                return pl

            def retention(lidx, last=False, pre_hc=None):
                if pre_hc is None:
                    roll_stats()
                    st = stats["cur"]
                else:
                    # input is already mean-0/var-1 (post final LN): the
                    # pre-LN is the identity; only inner-LN stats live here.
                    st = stps.tile([128, 512], f32, tag="st", name="stats")
                wmat = fetch_weights(lidx)
                w1_pre = None
                if lidx < N_WM:
                    # prefetch the FFN up-projection during the retention so
                    # its first LDWEIGHTS never waits on HBM
                    w1_pre = []
                    for c in range(PT):
                        wt = wpool.tile([128, 2048], bf16, tag="wmat",
                                        name=f"w1_{c}")
                        nc.sync.dma_start(out=wt[:], in_=W1T[:, c, :])
                        w1_pre.append(wt)
                bg_sb = wsmall.tile([128, 4], f32, tag="bgc")
                nc.sync.dma_start(out=bg_sb[:], in_=BG[lidx])
                bo_sb = wsmall.tile([128, 4], f32, tag="boc")
                nc.sync.dma_start(out=bo_sb[:], in_=BO[lidx])

                g_b = None
                if pre_hc is None:
                    # pre-LN: negmu from packed sums, center residual into hc
                    negmu = spool.tile([1, 512], bf16, tag="tiny", name="negmu")
                    nc.vector.tensor_scalar_mul(negmu[:], st[0:1, :], -1.0 / D)
                    plane = rpsum.tile([128, 512], f32, tag="rp", name="nmplane")
                    nc.tensor.matmul(plane[:], onesr_sb[:], negmu[:],
                                     start=True, stop=True)
                    hc = []
                    for c in range(PT):
                        t = hcpool.tile([128, 512], bf16, tag=f"hc{c}")
                        nc.vector.tensor_add(t[:], ht[c][:], plane[:])
                        hc.append(t)
                    rsk_sb = spool.tile([128, 4], f32, tag="rsk")
                else:
                    hc = pre_hc
                    rsk_sb = dkc_sb

                qt, kt, vn, gt = [], [], [], []

                def proj_block(ip, dest):
                    pss = [psum.tile([128, 512], f32, tag="big", name=f"pss{ip}_{i}")
                           for i in range(PT)]
                    for c in range(PT):
                        for j in range(PT):
                            if ip == 2:
                                nc.tensor.matmul(
                                    pss[j][:], hc[c][:, 128 * j : 128 * (j + 1)],
                                    wmat[2][:, c * 512 : (c + 1) * 512],
                                    start=(c == 0), stop=(c == PT - 1))
                            else:
                                nc.tensor.matmul(
                                    pss[j][:],
                                    wmat[ip][:, c * 512 + 128 * j : c * 512 + 128 * (j + 1)],
                                    hc[c][:], start=(c == 0), stop=(c == PT - 1))
                    for j in range(PT):
                        t = apool.tile([128, 512], bf16, tag=f"proj{ip}_{j}")
                        if ip == 0:
                            nc.vector.tensor_copy(t[:], pss[j][:])
                        elif ip == 1:
                            nc.scalar.copy(t[:], pss[j][:])
                        elif ip == 2:
                            nc.scalar.activation(t[:], pss[j][:], AF.Identity,
                                                 scale=rsk_sb[:, j : j + 1])
                        elif g_b is None:
                            nc.scalar.activation(t[:], pss[j][:], AF.Sigmoid,
                                                 bias=bg_sb[:, j : j + 1])
                        else:
                            tg = grpool.tile([128, 512], bf16, tag="gtmp")
                            nc.vector.tensor_mul(tg[:], pss[j][:], g_b[:])
                            nc.scalar.activation(t[:], tg[:], AF.Sigmoid,
                                                 bias=bg_sb[:, j : j + 1])
                        dest.append(t)

                proj_block(0, qt)
                if pre_hc is None:
                    # defer the ssq matmuls + rstd chain until the PE has the
                    # Q mains in flight; rsk transpose waits past K mains.
                    issue_ssq()
                    r = rstd_row(st[32:33, :], negmu, "pre")
                    r2 = spool.tile([1, 512], bf16, tag="tiny", name="r2")
                    nc.vector.tensor_mul(r2[:], r[:], r[:])
                    ks = spool.tile([1, 512], bf16, tag="tiny", name="ks")
                    nc.vector.tensor_mul(ks[:], r2[:], dk_sb[:])
                    g_b = pbcast(r, "gb")

                proj_block(1, kt)

                if pre_hc is None:
                    # ks row -> per-partition columns via K=1 transposes
                    rskp = rpsum.tile([128, 4], f32, tag="rp", name="rskp")
                    for j in range(PT):
                        nc.tensor.matmul(rskp[:, j : j + 1],
                                         ks[:, 128 * j : 128 * (j + 1)],
                                         onesr_sb[0:1, 0:1], start=True, stop=True)
                    nc.vector.tensor_copy(rsk_sb[:], rskp[:])

                proj_block(3, gt)
                proj_block(2, vn)

                # scores + AV, head pairs on row/col groups
                ret_sb = []
                for jt in range(PT):
                    rp = rpsum.tile([128, 512], f32, tag="rp", name=f"rp{jt}")
                    at_tiles = {}

                    def issue_sc(k_t):
                        cs = 128 * k_t
                        npr = 512 - cs
                        for hh in range(2):
                            r0 = 64 * hh
                            sc = psum.tile([128, 512], f32, tag="big",
                                           name=f"sc{k_t}_{hh}")
                            nc.tensor.matmul(
                                sc[:, 0:npr],
                                kt[jt][r0 : r0 + 64, cs : cs + 128],
                                qt[jt][r0 : r0 + 64, cs : 512],
                                start=True, stop=True)
                            at = atpool.tile([128, 512], bf16, tag="at")
                            nc.vector.tensor_mul(
                                at[:, 0:128], sc[:, 0:128], mskd_sb[:])
                            if npr > 128:
                                if k_t == 0:
                                    nc.vector.tensor_copy(at[:, 128:npr],
                                                          sc[:, 128:npr])
                                else:
                                    nc.scalar.copy(at[:, 128:npr], sc[:, 128:npr])
                            at_tiles[(hh, k_t)] = at

                    def issue_av(k_t):
                        cs = 128 * k_t
                        npr = 512 - cs
                        for hh in range(2):
                            h = 2 * jt + hh
                            nc.tensor.matmul(
                                rp[64 * hh : 64 * hh + 64, cs : 512],
                                vn[k_t][:, 64 * h : 64 * (h + 1)],
                                at_tiles[(hh, k_t)][:, 0:npr],
                                start=(k_t == 0), stop=(k_t == CT - 1),
                                skip_group_check=True)

                    # software pipeline: scores run two k-blocks ahead of the
                    # AV accumulation, giving the at drains slack and freeing
                    # score psum banks early.
                    issue_sc(0)
                    issue_sc(1)
                    issue_av(0)
                    issue_sc(2)
                    issue_av(1)
                    issue_sc(3)
                    issue_av(2)
                    issue_av(3)
                    rs = retpool.tile([128, 512], bf16, tag=f"ret{jt}")
                    nc.scalar.copy(rs[:], rp[:])
                    ret_sb.append(rs)
                    s2 = sqpool.tile([128, 512], bf16, tag=f"sq2{jt}")
                    nc.vector.tensor_mul(s2[:], rs[:], rs[:])
                    nc.tensor.matmul(st[0:1, :], onesc_sb[:], rs[:],
                                     start=(jt == 0), stop=(jt == PT - 1),
                                     skip_group_check=True)
                    nc.tensor.matmul(st[32:33, :], onesc_sb[:], s2[:],
                                     start=(jt == 0), stop=(jt == PT - 1),
                                     skip_group_check=True)

                # inner LN
                negmu2 = spool.tile([1, 512], bf16, tag="tiny", name="negmu2")
                nc.vector.tensor_scalar_mul(negmu2[:], st[0:1, :], -1.0 / D)
                nm2p = rpsum.tile([128, 512], f32, tag="rp", name="nm2p")
                nc.tensor.matmul(nm2p[:], onesr_sb[:], negmu2[:],
                                 start=True, stop=True)
                rB = rstd_row(st[32:33, :], negmu2, "inn")
                rstd2_b = pbcast(rB, "rstd2")

                gret = []
                for j in range(PT):
                    tmpc = odpool.tile([128, 512], bf16, tag="odb")
                    nc.vector.tensor_add(tmpc[:], ret_sb[j][:], nm2p[:])
                    gr = grpool.tile([128, 512], bf16, tag="gret")
                    nc.vector.tensor_mul(gr[:], tmpc[:], gt[j][:])
                    gret.append(gr)

                # O mains + rstd2/bias/residual, j-outer; next-block stats
                # (sums + Pool squares) interleave via set_ht.
                want_stats = (not last) and (lidx >= N_WM)
                shadow = lidx < N_WM
                for j in range(PT):
                    p1 = psum.tile([128, 512], f32, tag="big", name=f"p1s{j}")
                    for c in range(PT):
                        nc.tensor.matmul(
                            p1[:],
                            wmat[4][:, c * 512 + 128 * j : c * 512 + 128 * (j + 1)],
                            gret[c][:], start=(c == 0), stop=(c == PT - 1))
                    a = odpool.tile([128, 512], f32, tag="oda")
                    nc.vector.tensor_mul(a[:], p1[:], rstd2_b[:])
                    hn = hpool.tile([128, 512], f32r, tag=f"ht{j}")
                    nc.vector.scalar_tensor_tensor(hn[:], a[:], bo_sb[:, j : j + 1],
                                                   ht[j][:], ALU.add, ALU.add)
                    set_ht(j, hn, shadow=shadow, want_stats=want_stats)
                if want_stats:
                    issue_sums()
                return w1_pre

            def ffn(w1_sb):
                # stats for the successor block were accumulated by the
                # preceding retention's set_ht; this FFN does not read them.
                f2ps = [psum.tile([128, 512], f32, tag="big", name=f"f2ps{i}")
                        for i in range(PT)]
                for t in range(16):
                    p = psum.tile([128, 512], f32, tag="big")
                    for c in range(PT):
                        nc.tensor.matmul(
                            p[:], w1_sb[c][:, 128 * t : 128 * (t + 1)], ht_b[c][:],
                            start=(c == 0), stop=(c == PT - 1))
                    f1 = f1pool.tile([128, 512], bf16, tag="f1")
                    nc.scalar.activation(f1[:], p[:], AF.Gelu, bias=b1_sb[:, t : t + 1])
                    w2s = w2pool.tile([128, 512], bf16, tag="w2s")
                    nc.sync.dma_start(out=w2s[:], in_=W2T[:, t * 512 : (t + 1) * 512])
                    for j in range(PT):
                        nc.tensor.matmul(
                            f2ps[j][:], w2s[:, 128 * j : 128 * (j + 1)],
                            f1[:], start=(t == 0), stop=(t == 15))
                for j in range(PT):
                    hn = hpool.tile([128, 512], f32r, tag=f"ht{j}")
                    nc.vector.scalar_tensor_tensor(hn[:], f2ps[j][:], b2_sb[:, j : j + 1],
                                                   ht[j][:], ALU.add, ALU.add)
                    set_ht(j, hn)
                issue_sums()

            # world model layers. A wm retention's own set_ht feeds the
            # stats of the block after the FFN, and the FFN's set_ht feeds
            # the next retention / final LN: stats flow block-to-block.
            for l in range(N_WM):
                w1p = retention(l)
                ffn(w1p)

            # final LN of world model (stats packed by the last ffn).
            # wm_onw == 1 and wm_onb == 0 (asserted host-side), so this is a
            # plain LayerNorm: its output is mean-0/var-1 and the first core
            # retention's pre-LN becomes the identity -> feed hc directly.
            roll_stats()
            st = stats["cur"]
            issue_ssq()
            negmuf = spool.tile([1, 512], bf16, tag="tiny", name="negmuf")
            nc.vector.tensor_scalar_mul(negmuf[:], st[0:1, :], -1.0 / D)
            rf = rstd_row(st[32:33, :], negmuf, "fin")
            nmr = spool.tile([1, 512], bf16, tag="tiny", name="nmr")
            nc.vector.tensor_mul(nmr[:], negmuf[:], rf[:])
            rfp = psum.tile([128, 512], f32, tag="big", name="rfplane")
            nc.tensor.matmul(rfp[:], onesr_sb[:], rf[:], start=True, stop=True)
            nmrp = psum.tile([128, 512], f32, tag="big", name="nmrplane")
            nc.tensor.matmul(nmrp[:], onesr_sb[:], nmr[:], start=True, stop=True)
            pre_hc = []
            for j in range(PT):
                t1 = odpool.tile([128, 512], f32, tag="oda")
                nc.vector.tensor_mul(t1[:], ht[j][:], rfp[:])
                hn = hpool.tile([128, 512], f32r, tag=f"ht{j}")
                nc.vector.tensor_add(hn[:], t1[:], nmrp[:])
                set_ht(j, hn, want_stats=False)
                hcb = hcpool.tile([128, 512], bf16, tag=f"hc{j}")
                nc.scalar.copy(hcb[:], hn[:])
                pre_hc.append(hcb)

            # retention core layers
            retention(N_WM, pre_hc=pre_hc)
            for l in range(N_WM + 1, NL):
                retention(l, last=(l == NL - 1))

            for j in range(PT):
                nc.sync.dma_start(out=HOUT[j], in_=ht[j][:])

    nc.compile()
    return nc


def _host_prep(inputs):
    """Fold weights host-side; returns the shared in_map dict (no xt)."""
    import ml_dtypes
    bf = ml_dtypes.bfloat16
    g = {k: np.asarray(v, dtype=np.float32) for k, v in inputs.items()}

    def layer_params(l):
        if l < N_WM:
            pre = "wm_"
            i = l
        else:
            pre = "co_"
            i = l - N_WM
        return {n: g[pre + n][i] for n in
                ("wq", "bq", "wk", "bk", "wv", "bv", "wg", "bg", "wo", "bo",
                 "lnw", "lnb", "prew", "preb")}

    wst = np.zeros((NL, 5, 128, 2048), np.float32)
    bgc = np.zeros((NL, 128, 4), np.float32)
    boc = np.zeros((NL, 128, 4), np.float32)
    for l in range(NL):
        p = layer_params(l)
        wq = p["prew"][:, None] * p["wq"]
        wk = p["prew"][:, None] * p["wk"]
        wv = p["prew"][:, None] * p["wv"]
        wg = p["prew"][:, None] * p["wg"]
        wo = p["lnw"][:, None] * p["wo"]
        # biases bq~ = bq + preb @ wq must be zero for this folded fast path
        for nm, w in (("bq", p["wq"]), ("bk", p["wk"]), ("bv", p["wv"])):
            bb = p[nm] + p["preb"] @ w
            assert np.abs(bb).max() == 0.0, f"nonzero {nm} not supported"
        assert np.abs(p["lnb"]).max() == 0.0, "nonzero lnb not supported"
        bgf = p["bg"] + p["preb"] @ p["wg"]
        wst[l, 0] = _lhsT_layout(wq)
        wst[l, 1] = _lhsT_layout(wk)
        wst[l, 2] = _lhsT_layout(wv)
        wst[l, 3] = _lhsT_layout(wg)
        wst[l, 4] = _lhsT_layout(wo)
        bgc[l] = bgf.reshape(4, 128).T
        boc[l] = p["bo"].reshape(4, 128).T

    inw = _lhsT_layout(g["in_w"])
    inb = g["in_b"].reshape(4, 128).T.copy()
    w1t = _lhsT_layout(g["ffn_w1"]).reshape(128, 4, 2048)
    w2t = _lhsT_layout(g["ffn_w2"])  # [128, 16*512]
    b1c = g["ffn_b1"].reshape(16, 128).T.copy()
    b2c = g["ffn_b2"].reshape(4, 128).T.copy()
    assert np.all(g["wm_onw"] == 1.0), "non-unit wm_onw not supported"
    assert np.all(g["wm_onb"] == 0.0), "nonzero wm_onb not supported"

    q = np.arange(S, dtype=np.float64)
    dk = (DECAY ** (-q)).astype(np.float32).reshape(1, 512)
    mskd = np.triu(np.ones((128, 128), np.float32))

    return {
        "inw": inw.astype(bf), "inb": inb, "wst": wst.astype(bf),
        "bg": bgc, "bo": boc,
        "w1t": np.ascontiguousarray(w1t).astype(bf), "w2t": w2t.astype(bf),
        "b1c": b1c, "b2c": b2c,
        "dk": dk.astype(bf), "dkc": dk.reshape(4, 128).T.copy(), "mskd": mskd,
        "onesc": np.ones((128, 1), np.float32).astype(bf),
        "onesf": np.ones((128, 1), np.float32),
        "onesr": np.ones((1, 128), np.float32).astype(bf),
    }


def kernel(**inputs):
    from concourse.bass_utils import run_bass_kernel_spmd
    import ml_dtypes

    if "nc" not in _CACHE:
        _CACHE["nc"] = _build_program()
    nc = _CACHE["nc"]

    shared = _host_prep(inputs)
    x = np.asarray(inputs["x"], dtype=np.float32)
    in_maps = []
    for b in range(B):
        xt = np.ascontiguousarray(
            x[b].T.reshape(3, 128, 512).transpose(1, 0, 2)).astype(ml_dtypes.bfloat16)
        m = dict(shared)
        m["xt"] = xt
        in_maps.append(m)

    res = run_bass_kernel_spmd(nc, in_maps, list(range(B)))
    out = np.empty((B, S, D), np.float32)
    for b in range(B):
        hout = res.results[b]["hout"]  # [4,128,512] = ht tiles (transposed h)
        out[b] = hout.reshape(512, 512).T
    return out


# revision 21
# speedup vs baseline: 1.0842x; 1.0239x over previous
"""HaciCognitiveNet Trainium2 kernel, v3 (centered activations, packed stats).

Data-parallel over batch: B=8 -> one batch element per NeuronCore.
Activations live TRANSPOSED on-chip ([D, S], D on partitions).

Changes vs v2.1:
  - Rank-1 mean-correction matmuls (16/layer, 512 rows each) are GONE.
    The pre-LN mean is subtracted once: hc[c] = ht[c] + (-mu) broadcast,
    4 DVE ops/layer, and all four Q/K/V/G projections consume the
    centered hc tiles (V uses them as lhsT, so it centers for free).
  - LN stats (sums/ssq then sums2/ssq2) accumulate into ONE PSUM bank
    at partitions 0/32 (inner-LN rows reuse the pre-LN rows after their
    last read; M=1 matmuls with tile_position cols), freeing
    a bank: the main psum pool grows to 6 bufs, easing the Q->K->V->G
    drain backpressure that stalled the PE.
  - Stats matmuls for block l+1 issue during block l's residual drains
    (sums in set_ht); the ssq matmuls are deferred until after the next
    layer's Q mains so the PE never waits on the squares.
  - Squares for ssq run on the otherwise-idle Pool engine; rstd/gate/nm2
    planes broadcast there too (gpsimd.partition_broadcast) instead of
    PE matmul + Act drain. Only the latency-critical negmu plane keeps
    the K=1 PE broadcast (rpsum rotation: plane -> rskp -> rp x4).
  - bf16 residual shadows (ht_b) only exist where the FFN consumes them
    (after wm retentions); everywhere else DVE reads the f32r residual.
  - Drains split across engines: Q + G-premult + O + score diagonals on
    DVE, K/V/G-sigmoid/ret/score off-diagonals on Act.
"""

import numpy as np

B, S, DIN, D, H, FF = 8, 512, 384, 512, 8, 2048
DH = D // H
N_WM, N_CORE = 2, 4
NL = N_WM + N_CORE
DECAY = 0.99
EPS = 1e-5
PT = D // 128   # 4 partition tiles of the model dim
CT = S // 128   # 4 tiles of the sequence dim

_CACHE = {}


def _lhsT_layout(w):
    """[K, M] weight -> SBUF lhsT tile layout [128, (K//128)*M]."""
    k, m = w.shape
    c = k // 128
    return np.ascontiguousarray(
        w.reshape(c, 128, m).transpose(1, 0, 2).reshape(128, c * m)
    ).astype(np.float32)


def _build_program():
    import concourse.bass as bass
    import concourse.tile as tile
    from concourse import mybir, bacc
    from contextlib import ExitStack

    f32 = mybir.dt.float32
    f32r = mybir.dt.float32r
    bf16 = mybir.dt.bfloat16
    AF = mybir.ActivationFunctionType
    ALU = mybir.AluOpType

    nc = bacc.Bacc("TRN2", target_bir_lowering=False, debug=False)

    # Make Ln and Exp resolve to the single combined table set so each
    # LayerNorm chain pays one ACT_TABLE_LOAD instead of two.
    from concourse.hw_specs import get_activation_tables
    _tabs = get_activation_tables(nc.m.arch)
    for _name, _set in _tabs.items():
        if _name != "natural_log_exp_and_others":
            _set.discard(AF.Ln)
            _set.discard(AF.Exp)

    XT = nc.dram_tensor("xt", [128, 3, 512], bf16, kind="ExternalInput").ap()
    INW = nc.dram_tensor("inw", [128, 3 * 512], bf16, kind="ExternalInput").ap()
    INB = nc.dram_tensor("inb", [128, 4], f32, kind="ExternalInput").ap()
    WST = nc.dram_tensor("wst", [NL, 5, 128, 2048], bf16, kind="ExternalInput").ap()
    BG = nc.dram_tensor("bg", [NL, 128, 4], f32, kind="ExternalInput").ap()
    BO = nc.dram_tensor("bo", [NL, 128, 4], f32, kind="ExternalInput").ap()
    W1T = nc.dram_tensor("w1t", [128, 4, 2048], bf16, kind="ExternalInput").ap()
    W2T = nc.dram_tensor("w2t", [128, 16 * 512], bf16, kind="ExternalInput").ap()
    B1C = nc.dram_tensor("b1c", [128, 16], f32, kind="ExternalInput").ap()
    B2C = nc.dram_tensor("b2c", [128, 4], f32, kind="ExternalInput").ap()
    DK = nc.dram_tensor("dk", [1, 512], bf16, kind="ExternalInput").ap()
    DKC = nc.dram_tensor("dkc", [128, 4], f32, kind="ExternalInput").ap()
    MSKD = nc.dram_tensor("mskd", [128, 128], f32, kind="ExternalInput").ap()
    ONESC = nc.dram_tensor("onesc", [128, 1], bf16, kind="ExternalInput").ap()
    ONESF = nc.dram_tensor("onesf", [128, 1], f32r, kind="ExternalInput").ap()
    ONESR = nc.dram_tensor("onesr", [1, 128], bf16, kind="ExternalInput").ap()
    HOUT = nc.dram_tensor("hout", [4, 128, 512], f32r, kind="ExternalOutput").ap()

    with tile.TileContext(nc) as tc:
        with ExitStack() as ctx:
            consts = ctx.enter_context(tc.tile_pool(name="consts", bufs=1))
            wpool = ctx.enter_context(tc.tile_pool(name="wpool", bufs=10))
            w2pool = ctx.enter_context(tc.tile_pool(name="w2pool", bufs=4))
            wsmall = ctx.enter_context(tc.tile_pool(name="wsmall", bufs=2))
            hpool = ctx.enter_context(tc.tile_pool(name="hpool", bufs=2))
            hbpool = ctx.enter_context(tc.tile_pool(name="hbpool", bufs=2))
            hcpool = ctx.enter_context(tc.tile_pool(name="hcpool", bufs=2))
            apool = ctx.enter_context(tc.tile_pool(name="apool", bufs=1))
            atpool = ctx.enter_context(tc.tile_pool(name="atpool", bufs=8))
            spool = ctx.enter_context(tc.tile_pool(name="spool", bufs=8))
            sqpool = ctx.enter_context(tc.tile_pool(name="sqpool", bufs=2))
            grpool = ctx.enter_context(tc.tile_pool(name="grpool", bufs=5))
            retpool = ctx.enter_context(tc.tile_pool(name="retpool", bufs=1))
            plpool = ctx.enter_context(tc.tile_pool(name="plpool", bufs=3))
            odpool = ctx.enter_context(tc.tile_pool(name="odpool", bufs=2))
            f1pool = ctx.enter_context(tc.tile_pool(name="f1pool", bufs=3))
            psum = ctx.enter_context(tc.tile_pool(name="psum", bufs=6, space="PSUM"))
            stps = ctx.enter_context(tc.tile_pool(name="stps", bufs=1, space="PSUM"))
            rpsum = ctx.enter_context(tc.tile_pool(name="rpsum", bufs=1, space="PSUM"))

            ht = [None] * PT
            ht_b = [None] * PT
            stats = {"cur": None, "cur_sq": None, "nxt": None, "nxt_sq": None}

            def set_ht(j, hn, shadow=False, want_stats=True, bf_sums=False):
                """Residual tile update + next-block LN stats. bf_sums: make
                a bf16 shadow on the (idle-at-this-point) Act engine and use
                it for the sums matmul, dodging the f32r self-loading weight
                stall. Used at core-retention tails; FFN/inproj tails keep
                f32r sums because Act is busy with gelu there."""
                ht[j] = hn
                if shadow or bf_sums:
                    hb = hbpool.tile([128, 512], bf16, tag=f"htb{j}")
                    nc.scalar.copy(hb[:], hn[:])
                    ht_b[j] = hb
                if want_stats:
                    if j == 0:
                        stats["nxt"] = stps.tile([128, 512], f32, tag="st",
                                                 name="stats")
                        stats["nxt_sq"] = [None] * PT
                        stats["nxt_hn"] = [None] * PT
                        stats["nxt_bf"] = bf_sums
                    stats["nxt_hn"][j] = ht_b[j] if bf_sums else hn
                    sq = sqpool.tile([128, 512], bf16, tag=f"sqs{j}")
                    nc.vector.tensor_mul(sq[:], hn[:], hn[:])
                    stats["nxt_sq"][j] = sq

            def issue_sums():
                """sums matmuls, batched consecutively after the residual
                loop so weight loads never stall the O-mains pipeline."""
                ones = onesc_sb if stats["nxt_bf"] else onesf_sb
                for j in range(PT):
                    nc.tensor.matmul(stats["nxt"][0:1, :], ones[:],
                                     stats["nxt_hn"][j][:],
                                     start=(j == 0), stop=(j == PT - 1),
                                     skip_group_check=True)

            def roll_stats():
                stats["cur"] = stats["nxt"]
                stats["cur_sq"] = stats["nxt_sq"]
                stats["nxt"] = None
                stats["nxt_sq"] = None

            def issue_ssq():
                """ssq matmuls into the current stats tile, partition 32."""
                st = stats["cur"]
                for j in range(PT):
                    nc.tensor.matmul(st[32:33, :], onesc_sb[:],
                                     stats["cur_sq"][j][:],
                                     start=(j == 0), stop=(j == PT - 1),
                                     skip_group_check=True)

            def fetch_weights(lidx):
                wmat = []
                for i in range(5):
                    wt = wpool.tile([128, 2048], bf16, tag="wmat",
                                    name=f"wm{lidx}_{i}")
                    nc.sync.dma_start(out=wt[:], in_=WST[lidx, i])
                    wmat.append(wt)
                return wmat

            # ---- input projection: ht = (x @ in_w + in_b)^T ----
            inctx = ExitStack()
            inpool = inctx.enter_context(tc.tile_pool(name="inpool", bufs=1))
            xt_sb = inpool.tile([128, 3, 512], bf16)
            nc.sync.dma_start(out=xt_sb[:], in_=XT[:])
            inw_sb = inpool.tile([128, 3 * 512], bf16)
            nc.sync.dma_start(out=inw_sb[:], in_=INW[:])
            inb_sb = inpool.tile([128, 4], f32)
            nc.sync.dma_start(out=inb_sb[:], in_=INB[:])
            # ---- consts ----
            dk_sb = consts.tile([1, 512], bf16)
            nc.sync.dma_start(out=dk_sb[:], in_=DK[:])
            mskd_sb = consts.tile([128, 128], f32)
            nc.sync.dma_start(out=mskd_sb[:], in_=MSKD[:])
            onesc_sb = consts.tile([128, 1], bf16)
            nc.sync.dma_start(out=onesc_sb[:], in_=ONESC[:])
            onesf_sb = consts.tile([128, 1], f32r)
            nc.sync.dma_start(out=onesf_sb[:], in_=ONESF[:])
            onesr_sb = consts.tile([1, 128], bf16)
            nc.sync.dma_start(out=onesr_sb[:], in_=ONESR[:])
            dkc_sb = consts.tile([128, 4], f32)
            nc.sync.dma_start(out=dkc_sb[:], in_=DKC[:])
            b1_sb = consts.tile([128, 16], f32)
            nc.sync.dma_start(out=b1_sb[:], in_=B1C[:])
            b2_sb = consts.tile([128, 4], f32)
            nc.sync.dma_start(out=b2_sb[:], in_=B2C[:])

            for j in range(PT):
                p = psum.tile([128, 512], f32, tag="big")
                for c in range(3):
                    nc.tensor.matmul(
                        p[:], inw_sb[:, c * 512 + 128 * j : c * 512 + 128 * (j + 1)],
                        xt_sb[:, c, :], start=(c == 0), stop=(c == 2))
                hj = hpool.tile([128, 512], f32r, tag=f"ht{j}")
                nc.scalar.activation(hj[:], p[:], AF.Identity, bias=inb_sb[:, j : j + 1])
                set_ht(j, hj)
            issue_sums()
            inctx.close()

            def rstd_row(ssq_ps, negmu_b, name):
                """rstd = Exp(-0.5*Ln(ssq/D - mu^2 + eps)) -> bf16 [1,512]."""
                m2 = spool.tile([1, 512], f32, tag="tiny", name=f"m2{name}")
                nc.vector.tensor_mul(m2[:], negmu_b[:], negmu_b[:])
                m2e = spool.tile([1, 512], f32, tag="tiny", name=f"m2e{name}")
                nc.vector.tensor_scalar(m2e[:], m2[:], 1.0, -EPS, ALU.mult, ALU.add)
                w32 = spool.tile([1, 512], f32, tag="tiny", name=f"w32{name}")
                nc.vector.scalar_tensor_tensor(w32[:], ssq_ps[:], 1.0 / D, m2e[:],
                                               ALU.mult, ALU.subtract)
                lnw = spool.tile([1, 512], f32, tag="tiny", name=f"lnw{name}")
                nc.scalar.activation(lnw[:], w32[:], AF.Ln)
                r = spool.tile([1, 512], bf16, tag="tiny", name=f"r{name}")
                nc.scalar.activation(r[:], lnw[:], AF.Exp, scale=-0.5)
                return r

            def pbcast(row, name):
                """[1,512] bf16 row -> [128,512] bf16 plane on the Pool engine."""
                pl = plpool.tile([128, 512], bf16, tag="plane", name=f"pl{name}")
                nc.gpsimd.partition_broadcast(pl[:], row[:])
                return pl

            def retention(lidx, last=False, pre_hc=None):
                if pre_hc is None:
                    roll_stats()
                    st = stats["cur"]
                else:
                    # input is already mean-0/var-1 (post final LN): the
                    # pre-LN is the identity; only inner-LN stats live here.
                    st = stps.tile([128, 512], f32, tag="st", name="stats")
                wmat = fetch_weights(lidx)
                w1_pre = None
                if lidx < N_WM:
                    # prefetch the FFN up-projection during the retention so
                    # its first LDWEIGHTS never waits on HBM
                    w1_pre = []
                    for c in range(PT):
                        wt = wpool.tile([128, 2048], bf16, tag="wmat",
                                        name=f"w1_{c}")
                        nc.sync.dma_start(out=wt[:], in_=W1T[:, c, :])
                        w1_pre.append(wt)
                bg_sb = wsmall.tile([128, 4], f32, tag="bgc")
                nc.sync.dma_start(out=bg_sb[:], in_=BG[lidx])
                bo_sb = wsmall.tile([128, 4], f32, tag="boc")
                nc.sync.dma_start(out=bo_sb[:], in_=BO[lidx])

                g_b = None
                if pre_hc is None:
                    # pre-LN: negmu from packed sums, center residual into hc
                    negmu = spool.tile([1, 512], bf16, tag="tiny", name="negmu")
                    nc.vector.tensor_scalar_mul(negmu[:], st[0:1, :], -1.0 / D)
                    plane = rpsum.tile([128, 512], f32, tag="rp", name="nmplane")
                    nc.tensor.matmul(plane[:], onesr_sb[:], negmu[:],
                                     start=True, stop=True)
                    hc = []
                    for c in range(PT):
                        t = hcpool.tile([128, 512], bf16, tag=f"hc{c}")
                        nc.vector.tensor_add(t[:], ht[c][:], plane[:])
                        hc.append(t)
                    rsk_sb = spool.tile([128, 4], f32, tag="rsk")
                else:
                    hc = pre_hc
                    rsk_sb = dkc_sb

                qt, kt, vn, gt = [], [], [], []

                def proj_block(ip, dest):
                    pss = [psum.tile([128, 512], f32, tag="big", name=f"pss{ip}_{i}")
                           for i in range(PT)]
                    for c in range(PT):
                        for j in range(PT):
                            if ip == 2:
                                nc.tensor.matmul(
                                    pss[j][:], hc[c][:, 128 * j : 128 * (j + 1)],
                                    wmat[2][:, c * 512 : (c + 1) * 512],
                                    start=(c == 0), stop=(c == PT - 1))
                            else:
                                nc.tensor.matmul(
                                    pss[j][:],
                                    wmat[ip][:, c * 512 + 128 * j : c * 512 + 128 * (j + 1)],
                                    hc[c][:], start=(c == 0), stop=(c == PT - 1))
                    for j in range(PT):
                        t = apool.tile([128, 512], bf16, tag=f"proj{ip}_{j}")
                        if ip == 0:
                            nc.vector.tensor_copy(t[:], pss[j][:])
                        elif ip == 1:
                            nc.scalar.copy(t[:], pss[j][:])
                        elif ip == 2:
                            nc.scalar.activation(t[:], pss[j][:], AF.Identity,
                                                 scale=rsk_sb[:, j : j + 1])
                        elif g_b is None:
                            nc.scalar.activation(t[:], pss[j][:], AF.Sigmoid,
                                                 bias=bg_sb[:, j : j + 1])
                        else:
                            tg = grpool.tile([128, 512], bf16, tag="gtmp")
                            nc.vector.tensor_mul(tg[:], pss[j][:], g_b[:])
                            nc.scalar.activation(t[:], tg[:], AF.Sigmoid,
                                                 bias=bg_sb[:, j : j + 1])
                        dest.append(t)

                proj_block(0, qt)
                if pre_hc is None:
                    # defer the ssq matmuls + rstd chain until the PE has the
                    # Q mains in flight; rsk transpose waits past K mains.
                    issue_ssq()
                    r = rstd_row(st[32:33, :], negmu, "pre")
                    r2 = spool.tile([1, 512], bf16, tag="tiny", name="r2")
                    nc.vector.tensor_mul(r2[:], r[:], r[:])
                    ks = spool.tile([1, 512], bf16, tag="tiny", name="ks")
                    nc.vector.tensor_mul(ks[:], r2[:], dk_sb[:])
                    g_b = pbcast(r, "gb")

                proj_block(1, kt)

                if pre_hc is None:
                    # ks row -> per-partition columns via K=1 transposes
                    rskp = rpsum.tile([128, 4], f32, tag="rp", name="rskp")
                    for j in range(PT):
                        nc.tensor.matmul(rskp[:, j : j + 1],
                                         ks[:, 128 * j : 128 * (j + 1)],
                                         onesr_sb[0:1, 0:1], start=True, stop=True)
                    nc.vector.tensor_copy(rsk_sb[:], rskp[:])

                proj_block(3, gt)
                proj_block(2, vn)

                # scores + AV, head pairs on row/col groups
                ret_sb = []
                for jt in range(PT):
                    rp = rpsum.tile([128, 512], f32, tag="rp", name=f"rp{jt}")
                    at_tiles = {}

                    def issue_sc(k_t):
                        cs = 128 * k_t
                        npr = 512 - cs
                        for hh in range(2):
                            r0 = 64 * hh
                            sc = psum.tile([128, 512], f32, tag="big",
                                           name=f"sc{k_t}_{hh}")
                            nc.tensor.matmul(
                                sc[:, 0:npr],
                                kt[jt][r0 : r0 + 64, cs : cs + 128],
                                qt[jt][r0 : r0 + 64, cs : 512],
                                start=True, stop=True)
                            at = atpool.tile([128, 512], bf16, tag="at")
                            nc.vector.tensor_mul(
                                at[:, 0:128], sc[:, 0:128], mskd_sb[:])
                            if npr > 128:
                                if k_t == 0:
                                    nc.vector.tensor_copy(at[:, 128:npr],
                                                          sc[:, 128:npr])
                                else:
                                    nc.scalar.copy(at[:, 128:npr], sc[:, 128:npr])
                            at_tiles[(hh, k_t)] = at

                    def issue_av(k_t):
                        cs = 128 * k_t
                        npr = 512 - cs
                        for hh in range(2):
                            h = 2 * jt + hh
                            nc.tensor.matmul(
                                rp[64 * hh : 64 * hh + 64, cs : 512],
                                vn[k_t][:, 64 * h : 64 * (h + 1)],
                                at_tiles[(hh, k_t)][:, 0:npr],
                                start=(k_t == 0), stop=(k_t == CT - 1),
                                skip_group_check=True)

                    # software pipeline: scores run two k-blocks ahead of the
                    # AV accumulation, giving the at drains slack and freeing
                    # score psum banks early.
                    issue_sc(0)
                    issue_sc(1)
                    issue_av(0)
                    issue_sc(2)
                    issue_av(1)
                    issue_sc(3)
                    issue_av(2)
                    issue_av(3)
                    rs = retpool.tile([128, 512], bf16, tag=f"ret{jt}")
                    nc.scalar.copy(rs[:], rp[:])
                    ret_sb.append(rs)
                    s2 = sqpool.tile([128, 512], bf16, tag=f"sq2{jt}")
                    nc.vector.tensor_mul(s2[:], rs[:], rs[:])
                    nc.tensor.matmul(st[0:1, :], onesc_sb[:], rs[:],
                                     start=(jt == 0), stop=(jt == PT - 1),
                                     skip_group_check=True)
                    nc.tensor.matmul(st[32:33, :], onesc_sb[:], s2[:],
                                     start=(jt == 0), stop=(jt == PT - 1),
                                     skip_group_check=True)

                # inner LN
                negmu2 = spool.tile([1, 512], bf16, tag="tiny", name="negmu2")
                nc.vector.tensor_scalar_mul(negmu2[:], st[0:1, :], -1.0 / D)
                nm2p = rpsum.tile([128, 512], f32, tag="rp", name="nm2p")
                nc.tensor.matmul(nm2p[:], onesr_sb[:], negmu2[:],
                                 start=True, stop=True)
                rB = rstd_row(st[32:33, :], negmu2, "inn")
                rstd2_b = pbcast(rB, "rstd2")

                gret = []
                for j in range(PT):
                    tmpc = odpool.tile([128, 512], bf16, tag="odb")
                    nc.vector.tensor_add(tmpc[:], ret_sb[j][:], nm2p[:])
                    gr = grpool.tile([128, 512], bf16, tag="gret")
                    nc.vector.tensor_mul(gr[:], tmpc[:], gt[j][:])
                    gret.append(gr)

                # O mains + rstd2/bias/residual, j-outer; next-block stats
                # (sums + Pool squares) interleave via set_ht.
                want_stats = (not last) and (lidx >= N_WM)
                shadow = lidx < N_WM
                for j in range(PT):
                    p1 = psum.tile([128, 512], f32, tag="big", name=f"p1s{j}")
                    for c in range(PT):
                        nc.tensor.matmul(
                            p1[:],
                            wmat[4][:, c * 512 + 128 * j : c * 512 + 128 * (j + 1)],
                            gret[c][:], start=(c == 0), stop=(c == PT - 1))
                    a = odpool.tile([128, 512], f32, tag="oda")
                    nc.vector.tensor_mul(a[:], p1[:], rstd2_b[:])
                    hn = hpool.tile([128, 512], f32r, tag=f"ht{j}")
                    nc.vector.scalar_tensor_tensor(hn[:], a[:], bo_sb[:, j : j + 1],
                                                   ht[j][:], ALU.add, ALU.add)
                    set_ht(j, hn, shadow=shadow, want_stats=want_stats,
                           bf_sums=want_stats)
                if want_stats:
                    issue_sums()
                return w1_pre

            def ffn(w1_sb):
                # stats for the successor block were accumulated by the
                # preceding retention's set_ht; this FFN does not read them.
                f2ps = [psum.tile([128, 512], f32, tag="big", name=f"f2ps{i}")
                        for i in range(PT)]
                for t in range(16):
                    p = psum.tile([128, 512], f32, tag="big")
                    for c in range(PT):
                        nc.tensor.matmul(
                            p[:], w1_sb[c][:, 128 * t : 128 * (t + 1)], ht_b[c][:],
                            start=(c == 0), stop=(c == PT - 1))
                    f1 = f1pool.tile([128, 512], bf16, tag="f1")
                    nc.scalar.activation(f1[:], p[:], AF.Gelu, bias=b1_sb[:, t : t + 1])
                    w2s = w2pool.tile([128, 512], bf16, tag="w2s")
                    nc.sync.dma_start(out=w2s[:], in_=W2T[:, t * 512 : (t + 1) * 512])
                    for j in range(PT):
                        nc.tensor.matmul(
                            f2ps[j][:], w2s[:, 128 * j : 128 * (j + 1)],
                            f1[:], start=(t == 0), stop=(t == 15))
                for j in range(PT):
                    hn = hpool.tile([128, 512], f32r, tag=f"ht{j}")
                    nc.vector.scalar_tensor_tensor(hn[:], f2ps[j][:], b2_sb[:, j : j + 1],
                                                   ht[j][:], ALU.add, ALU.add)
                    set_ht(j, hn)
                issue_sums()

            # world model layers. A wm retention's own set_ht feeds the
            # stats of the block after the FFN, and the FFN's set_ht feeds
            # the next retention / final LN: stats flow block-to-block.
            for l in range(N_WM):
                w1p = retention(l)
                ffn(w1p)

            # final LN of world model (stats packed by the last ffn).
            # wm_onw == 1 and wm_onb == 0 (asserted host-side), so this is a
            # plain LayerNorm: its output is mean-0/var-1 and the first core
            # retention's pre-LN becomes the identity -> feed hc directly.
            roll_stats()
            st = stats["cur"]
            issue_ssq()
            negmuf = spool.tile([1, 512], bf16, tag="tiny", name="negmuf")
            nc.vector.tensor_scalar_mul(negmuf[:], st[0:1, :], -1.0 / D)
            rf = rstd_row(st[32:33, :], negmuf, "fin")
            nmr = spool.tile([1, 512], bf16, tag="tiny", name="nmr")
            nc.vector.tensor_mul(nmr[:], negmuf[:], rf[:])
            rfp = psum.tile([128, 512], f32, tag="big", name="rfplane")
            nc.tensor.matmul(rfp[:], onesr_sb[:], rf[:], start=True, stop=True)
            nmrp = psum.tile([128, 512], f32, tag="big", name="nmrplane")
            nc.tensor.matmul(nmrp[:], onesr_sb[:], nmr[:], start=True, stop=True)
            pre_hc = []
            for j in range(PT):
                t1 = odpool.tile([128, 512], f32, tag="oda")
                nc.vector.tensor_mul(t1[:], ht[j][:], rfp[:])
                hn = hpool.tile([128, 512], f32r, tag=f"ht{j}")
                nc.vector.tensor_add(hn[:], t1[:], nmrp[:])
                set_ht(j, hn, want_stats=False)
                hcb = hcpool.tile([128, 512], bf16, tag=f"hc{j}")
                nc.scalar.copy(hcb[:], hn[:])
                pre_hc.append(hcb)

            # retention core layers
            retention(N_WM, pre_hc=pre_hc)
            for l in range(N_WM + 1, NL):
                retention(l, last=(l == NL - 1))

            for j in range(PT):
                nc.sync.dma_start(out=HOUT[j], in_=ht[j][:])

    nc.compile()
    return nc


def _host_prep(inputs):
    """Fold weights host-side; returns the shared in_map dict (no xt)."""
    import ml_dtypes
    bf = ml_dtypes.bfloat16
    g = {k: np.asarray(v, dtype=np.float32) for k, v in inputs.items()}

    def layer_params(l):
        if l < N_WM:
            pre = "wm_"
            i = l
        else:
            pre = "co_"
            i = l - N_WM
        return {n: g[pre + n][i] for n in
                ("wq", "bq", "wk", "bk", "wv", "bv", "wg", "bg", "wo", "bo",
                 "lnw", "lnb", "prew", "preb")}

    wst = np.zeros((NL, 5, 128, 2048), np.float32)
    bgc = np.zeros((NL, 128, 4), np.float32)
    boc = np.zeros((NL, 128, 4), np.float32)
    for l in range(NL):
        p = layer_params(l)
        wq = p["prew"][:, None] * p["wq"]
        wk = p["prew"][:, None] * p["wk"]
        wv = p["prew"][:, None] * p["wv"]
        wg = p["prew"][:, None] * p["wg"]
        wo = p["lnw"][:, None] * p["wo"]
        # biases bq~ = bq + preb @ wq must be zero for this folded fast path
        for nm, w in (("bq", p["wq"]), ("bk", p["wk"]), ("bv", p["wv"])):
            bb = p[nm] + p["preb"] @ w
            assert np.abs(bb).max() == 0.0, f"nonzero {nm} not supported"
        assert np.abs(p["lnb"]).max() == 0.0, "nonzero lnb not supported"
        bgf = p["bg"] + p["preb"] @ p["wg"]
        wst[l, 0] = _lhsT_layout(wq)
        wst[l, 1] = _lhsT_layout(wk)
        wst[l, 2] = _lhsT_layout(wv)
        wst[l, 3] = _lhsT_layout(wg)
        wst[l, 4] = _lhsT_layout(wo)
        bgc[l] = bgf.reshape(4, 128).T
        boc[l] = p["bo"].reshape(4, 128).T

    inw = _lhsT_layout(g["in_w"])
    inb = g["in_b"].reshape(4, 128).T.copy()
    w1t = _lhsT_layout(g["ffn_w1"]).reshape(128, 4, 2048)
    w2t = _lhsT_layout(g["ffn_w2"])  # [128, 16*512]
    b1c = g["ffn_b1"].reshape(16, 128).T.copy()
    b2c = g["ffn_b2"].reshape(4, 128).T.copy()
    assert np.all(g["wm_onw"] == 1.0), "non-unit wm_onw not supported"
    assert np.all(g["wm_onb"] == 0.0), "nonzero wm_onb not supported"

    q = np.arange(S, dtype=np.float64)
    dk = (DECAY ** (-q)).astype(np.float32).reshape(1, 512)
    mskd = np.triu(np.ones((128, 128), np.float32))

    return {
        "inw": inw.astype(bf), "inb": inb, "wst": wst.astype(bf),
        "bg": bgc, "bo": boc,
        "w1t": np.ascontiguousarray(w1t).astype(bf), "w2t": w2t.astype(bf),
        "b1c": b1c, "b2c": b2c,
        "dk": dk.astype(bf), "dkc": dk.reshape(4, 128).T.copy(), "mskd": mskd,
        "onesc": np.ones((128, 1), np.float32).astype(bf),
        "onesf": np.ones((128, 1), np.float32),
        "onesr": np.ones((1, 128), np.float32).astype(bf),
    }


def kernel(**inputs):
    from concourse.bass_utils import run_bass_kernel_spmd
    import ml_dtypes

    if "nc" not in _CACHE:
        _CACHE["nc"] = _build_program()
    nc = _CACHE["nc"]

    shared = _host_prep(inputs)
    x = np.asarray(inputs["x"], dtype=np.float32)
    in_maps = []
    for b in range(B):
        xt = np.ascontiguousarray(
            x[b].T.reshape(3, 128, 512).transpose(1, 0, 2)).astype(ml_dtypes.bfloat16)
        m = dict(shared)
        m["xt"] = xt
        in_maps.append(m)

    res = run_bass_kernel_spmd(nc, in_maps, list(range(B)))
    out = np.empty((B, S, D), np.float32)
    for b in range(B):
        hout = res.results[b]["hout"]  # [4,128,512] = ht tiles (transposed h)
        out[b] = hout.reshape(512, 512).T
    return out
